# revision 3
# baseline (speedup 1.0000x reference)
"""Cross-attention kernel for 8 Trainium2 NeuronCores (Bass/Tile).

Sharding: data-parallel over (L, query-half). Core c handles batch l = c//2
and queries [(c%2)*1024, (c%2+1)*1024) of that batch. K/V for the full 2048
keys of batch l are computed on both cores of the pair (duplicated work, no
cross-core communication).

Per-core dataflow (matmuls in bf16 with f32 PSUM accumulation):
  qT[e, nq], kT[e, nk]   transposed projections (feature dim on partitions)
  v'[nk, h, 65]          v projection + a ones column per head (softmax denom)
  sT[nk, nq] = kT.T @ qT per head  -- scores transposed, keys on partitions;
                                      head pairs run row-packed on the PE
  attnT = exp(SCALE*sT + maskbias[nk])  one ACT op fuses scale+mask+exp+cast
  out'[65, nq] = v'.T @ attnT      rows 0..63: head out^T, row 64: denominator
  normalize: DVE reciprocal of row 64, broadcast across partitions via a
             tiny ones-block matmul, DVE multiply
  out = out_allT.T @ WoT + bo      final projection, bias via a K=1 matmul
"""

import numpy as np
import ml_dtypes
from contextlib import ExitStack

import concourse.bass as bass
import concourse.tile as tile
from concourse import bacc, mybir
from concourse.bass_utils import run_bass_kernel_spmd

L, N, D_IN = 4, 2048, 1024
H, DH = 8, 64
INNER = H * DH          # 512
D_OUT = D_IN
SCALE = DH ** -0.5      # 0.125
NQ = N // 2             # 1024 queries per core
NCORES = 8
DC = D_IN // 128        # 8 contraction chunks for the projections
EC = INNER // 128       # 4 feature chunks (= head pairs)
KC = N // 128           # 16 key chunks
NKB = N // 512          # 4 key 512-blocks
NQB = NQ // 512         # 2 query 512-blocks
MASK_NEG = -50.0

BF = mybir.dt.bfloat16
F32 = mybir.dt.float32
EXP = mybir.ActivationFunctionType.Exp


def _emit(ctx, tc, xT, xqT, wqT, wkT, wvT, woT, bo, maskb, out):
    nc = tc.nc

    const = ctx.enter_context(tc.tile_pool(name="const", bufs=1))
    big = ctx.enter_context(tc.tile_pool(name="big", bufs=1))
    attn_sb = ctx.enter_context(tc.tile_pool(name="attn_sb", bufs=4))
    norm_sb = ctx.enter_context(tc.tile_pool(name="norm_sb", bufs=3))
    out_sb = ctx.enter_context(tc.tile_pool(name="out_sb", bufs=2))
    ps_st = ctx.enter_context(tc.tile_pool(name="ps_st", bufs=2, space="PSUM"))
    ps_av = ctx.enter_context(tc.tile_pool(name="ps_av", bufs=3, space="PSUM"))
    ps_bc = ctx.enter_context(tc.tile_pool(name="ps_bc", bufs=1, space="PSUM"))

    # ---- inputs -> SBUF (kT-projection inputs first so the PE starts early)
    wk_s = const.tile([128, DC, INNER], BF)
    wq_s = const.tile([128, DC, INNER], BF)
    wv_s = const.tile([128, DC, INNER], BF)
    wo_s = const.tile([128, EC, D_OUT], BF)
    bo_s = const.tile([1, D_OUT], F32)
    maskb_s = const.tile([128, KC], F32)
    xT_s = big.tile([128, DC, N], BF)
    xqT_s = big.tile([128, DC, NQ], BF)
    for d in range(DC):
        nc.sync.dma_start(wk_s[:, d, :], wkT[d])
        nc.sync.dma_start(xT_s[:, d, :], xT[d])
    for d in range(DC):
        nc.sync.dma_start(wq_s[:, d, :], wqT[d])
        nc.sync.dma_start(xqT_s[:, d, :], xqT[d])
        nc.sync.dma_start(wv_s[:, d, :], wvT[d])
    nc.sync.dma_start(maskb_s, maskb)
    for j in range(EC):
        nc.sync.dma_start(wo_s[:, j, :], woT[j])
    nc.sync.dma_start(bo_s, bo)

    ones_row = const.tile([1, 128], F32)
    nc.vector.memset(ones_row, 1.0)

    kT_s = big.tile([128, EC, N], BF)
    qT_s = big.tile([128, EC, NQ], BF)
    vp_s = big.tile([128, KC, H, DH + 1], BF)
    nc.vector.memset(vp_s[:, :, :, DH], 1.0)
    out_allT = big.tile([128, EC, NQ], BF)

    def proj_kT(j, b):
        ps = ps_av.tile([128, 512], F32, tag="av", name=f"ps_k{j}{b}")
        for d in range(DC):
            nc.tensor.matmul(
                ps, wk_s[:, d, j * 128:(j + 1) * 128],
                xT_s[:, d, b * 512:(b + 1) * 512],
                start=(d == 0), stop=(d == DC - 1))
        nc.vector.tensor_copy(kT_s[:, j, b * 512:(b + 1) * 512], ps)

    def proj_qT(j, b):
        ps = ps_av.tile([128, 512], F32, tag="av", name=f"ps_q{j}{b}")
        for d in range(DC):
            nc.tensor.matmul(
                ps, wq_s[:, d, j * 128:(j + 1) * 128],
                xqT_s[:, d, b * 512:(b + 1) * 512],
                start=(d == 0), stop=(d == DC - 1))
        nc.vector.tensor_copy(qT_s[:, j, b * 512:(b + 1) * 512], ps)

    def proj_v(c):
        ps = ps_av.tile([128, 512], F32, tag="av", name=f"ps_v{c}")
        for d in range(DC):
            nc.tensor.matmul(
                ps, xT_s[:, d, c * 128:(c + 1) * 128], wv_s[:, d, :],
                start=(d == 0), stop=(d == DC - 1))
        nc.vector.tensor_copy(
            vp_s[:, c, :, 0:DH],
            ps.rearrange("p (h e) -> p h e", h=H))

    # kT/qT for the first head pair, then v'; the rest interleaves below.
    for b in range(NKB):
        proj_kT(0, b)
    for b in range(NQB):
        proj_qT(0, b)

    fills = {p: [("k", p + 1, b) for b in range(NKB)]
                + [("q", p + 1, b) for b in range(NQB)]
             for p in range(EC - 1)}

    def emit_fill(p, it):
        lst = fills.get(p)
        if not lst:
            return
        # spread the 6 groups over the 32 chunk iterations of this pair
        if it % 5 == 4 and lst:
            kind, j, b = lst.pop(0)
            (proj_kT if kind == "k" else proj_qT)(j, b)

    for p in range(EC):
        hA, hB = 2 * p, 2 * p + 1
        for qb in range(NQB):
            oA = ps_av.tile([128, 512], F32, tag="av", name=f"oA{p}{qb}")
            oB = ps_av.tile([128, 512], F32, tag="av", name=f"oB{p}{qb}")
            for c in range(KC):
                if p == 0 and qb == 0:
                    proj_v(c)
                sT = ps_st.tile([128, 1024], F32, tag="st", name="sT")
                nc.tensor.matmul(
                    sT[:, 0:512],
                    kT_s[0:64, p, c * 128:(c + 1) * 128],
                    qT_s[0:64, p, qb * 512:(qb + 1) * 512],
                    start=True, stop=True)
                nc.tensor.matmul(
                    sT[:, 512:1024],
                    kT_s[64:128, p, c * 128:(c + 1) * 128],
                    qT_s[64:128, p, qb * 512:(qb + 1) * 512],
                    start=True, stop=True)
                at = attn_sb.tile([128, 1024], BF, tag="at", name="at")
                nc.scalar.activation(at, sT, EXP,
                                     bias=maskb_s[:, c:c + 1], scale=SCALE)
                nc.tensor.matmul(oA[0:DH + 1, :], vp_s[:, c, hA, :],
                                 at[:, 0:512],
                                 start=(c == 0), stop=(c == KC - 1))
                nc.tensor.matmul(oB[0:DH + 1, :], vp_s[:, c, hB, :],
                                 at[:, 512:1024],
                                 start=(c == 0), stop=(c == KC - 1))
                emit_fill(p, qb * KC + c)

            # normalize: out_allT[head rows] = out' * (1/denominator)
            recipA = norm_sb.tile([1, 512], F32, tag="recipA", name="recipA")
            recipB = norm_sb.tile([1, 512], F32, tag="recipB", name="recipB")
            nc.vector.reciprocal(recipA, oA[DH:DH + 1, :])
            nc.vector.reciprocal(recipB, oB[DH:DH + 1, :])
            bc = ps_bc.tile([128, 512], F32, tag="bc", name="bc")
            nc.tensor.matmul(bc[0:64, :], ones_row[0:1, 0:64], recipA,
                             start=True, stop=True)
            nc.tensor.matmul(bc[64:128, :], ones_row[0:1, 0:64], recipB,
                             start=True, stop=True)
            bc_s = norm_sb.tile([128, 512], F32, tag="bcs", name="bc_s")
            nc.vector.tensor_copy(bc_s, bc)
            nc.vector.tensor_mul(
                out_allT[0:64, p, qb * 512:(qb + 1) * 512],
                oA[0:DH, :], bc_s[0:64, :])
            nc.vector.tensor_mul(
                out_allT[64:128, p, qb * 512:(qb + 1) * 512],
                oB[0:DH, :], bc_s[64:128, :])

    # ---- output projection (+ bias via K=1 matmul)
    for t in range(NQ // 128):
        of = out_sb.tile([128, D_OUT], F32, tag="of", name="of")
        for f in range(D_OUT // 512):
            po = ps_av.tile([128, 512], F32, tag="av", name=f"po{t}{f}")
            nc.tensor.matmul(po, ones_row, bo_s[0:1, f * 512:(f + 1) * 512],
                             start=True, stop=False)
            for j in range(EC):
                nc.tensor.matmul(
                    po, out_allT[:, j, t * 128:(t + 1) * 128],
                    wo_s[:, j, f * 512:(f + 1) * 512],
                    start=False, stop=(j == EC - 1))
            nc.vector.tensor_copy(of[:, f * 512:(f + 1) * 512], po)
        nc.sync.dma_start(out[t * 128:(t + 1) * 128, :], of)


def _build():
    nc = bacc.Bacc("TRN2", target_bir_lowering=False, debug=False,
                   num_devices=NCORES)
    aps = dict(
        xT=nc.dram_tensor("xT", [DC, 128, N], BF, kind="ExternalInput").ap(),
        xqT=nc.dram_tensor("xqT", [DC, 128, NQ], BF, kind="ExternalInput").ap(),
        wqT=nc.dram_tensor("wqT", [DC, 128, INNER], BF, kind="ExternalInput").ap(),
        wkT=nc.dram_tensor("wkT", [DC, 128, INNER], BF, kind="ExternalInput").ap(),
        wvT=nc.dram_tensor("wvT", [DC, 128, INNER], BF, kind="ExternalInput").ap(),
        woT=nc.dram_tensor("woT", [EC, 128, D_OUT], BF, kind="ExternalInput").ap(),
        bo=nc.dram_tensor("bo", [1, D_OUT], F32, kind="ExternalInput").ap(),
        maskb=nc.dram_tensor("maskb", [128, KC], F32, kind="ExternalInput").ap(),
        out=nc.dram_tensor("out", [NQ, D_OUT], F32, kind="ExternalOutput").ap(),
    )
    with tile.TileContext(nc) as tc:
        with ExitStack() as ctx:
            _emit(ctx, tc, **aps)
    nc.compile()
    return nc


_prog = None


def _get_prog():
    global _prog
    if _prog is None:
        _prog = _build()
    return _prog


def _make_in_maps(x, Wq, Wk, Wv, Wo, bo, mask):
    bf = ml_dtypes.bfloat16
    f32 = np.float32
    wqT = np.ascontiguousarray(Wq.T).astype(bf).reshape(DC, 128, INNER)
    wkT = np.ascontiguousarray(Wk.T).astype(bf).reshape(DC, 128, INNER)
    wvT = np.ascontiguousarray(Wv.T).astype(bf).reshape(DC, 128, INNER)
    woT = np.ascontiguousarray(Wo.T).astype(bf).reshape(EC, 128, D_OUT)
    bo2 = np.ascontiguousarray(bo).astype(f32).reshape(1, D_OUT)
    in_maps = []
    for c in range(NCORES):
        l, qh = c // 2, c % 2
        xTl = np.ascontiguousarray(x[l].T).astype(bf).reshape(DC, 128, N)
        xqT = np.ascontiguousarray(
            x[l, qh * NQ:(qh + 1) * NQ, :].T).astype(bf).reshape(DC, 128, NQ)
        mb = np.where(mask[l], 0.0, MASK_NEG).astype(f32)
        mb = np.ascontiguousarray(mb.reshape(KC, 128).T)
        in_maps.append(dict(xT=xTl, xqT=xqT, wqT=wqT, wkT=wkT, wvT=wvT,
                            woT=woT, bo=bo2, maskb=mb))
    return in_maps


def run(x, Wq, Wk, Wv, Wo, bo, mask, trace=False, tmpdir=None):
    nc = _get_prog()
    in_maps = _make_in_maps(x, Wq, Wk, Wv, Wo, bo, mask)
    res = run_bass_kernel_spmd(nc, in_maps, core_ids=list(range(NCORES)),
                               trace=trace, tmpdir=tmpdir)
    out = np.empty((L, N, D_OUT), np.float32)
    for c in range(NCORES):
        l, qh = c // 2, c % 2
        out[l, qh * NQ:(qh + 1) * NQ, :] = res.results[c]["out"]
    return out, res


def kernel(x, Wq, Wk, Wv, Wo, bo, mask):
    out, _ = run(np.asarray(x, np.float32), np.asarray(Wq, np.float32),
                 np.asarray(Wk, np.float32), np.asarray(Wv, np.float32),
                 np.asarray(Wo, np.float32), np.asarray(bo, np.float32),
                 np.asarray(mask))
    return out


# revision 7
# speedup vs baseline: 1.1086x; 1.1086x over previous
"""Cross-attention kernel for 8 Trainium2 NeuronCores (Bass/Tile).

Sharding: data-parallel over (L, query-half). Core c handles batch l = c//2
and queries [(c%2)*1024, (c%2+1)*1024) of that batch. K/V for the full 2048
keys of batch l are computed on both cores of the pair (duplicated work, no
cross-core communication).

Per-core dataflow (matmuls in bf16 with f32 PSUM accumulation):
  qT[e, nq], kT[e, nk]   transposed projections (feature dim on partitions)
  v'[nk, h, 65]          v projection + a ones column per head (softmax denom)
  sT[nk, nq] = kT.T @ qT per head  -- scores transposed, keys on partitions;
                                      head pairs run row-packed on the PE
  attnT = exp(SCALE*sT + maskbias[nk])  one ACT op fuses scale+mask+exp+cast
  out'[65, nq] = v'.T @ attnT      rows 0..63: head out^T, row 64: denominator
  normalize: DVE reciprocal of row 64, broadcast across partitions via a
             tiny ones-block matmul, DVE multiply
  out = out_allT.T @ WoT + bo      final projection, bias via a K=1 matmul
"""

import numpy as np
import ml_dtypes
from contextlib import ExitStack

import concourse.bass as bass
import concourse.tile as tile
from concourse import bacc, mybir
from concourse.bass_utils import run_bass_kernel_spmd

L, N, D_IN = 4, 2048, 1024
H, DH = 8, 64
INNER = H * DH          # 512
D_OUT = D_IN
SCALE = DH ** -0.5      # 0.125
NQ = N // 2             # 1024 queries per core
NCORES = 8
DC = D_IN // 128        # 8 contraction chunks for the projections
EC = INNER // 128       # 4 feature chunks (= head pairs)
KC = N // 128           # 16 key chunks
NKB = N // 512          # 4 key 512-blocks
NQB = NQ // 512         # 2 query 512-blocks
MASK_NEG = -50.0

BF = mybir.dt.bfloat16
F32 = mybir.dt.float32
EXP = mybir.ActivationFunctionType.Exp


def _emit(ctx, tc, xT, xqT, wqT, wkT, wvT, woT, bo, maskb, out):
    nc = tc.nc

    const = ctx.enter_context(tc.tile_pool(name="const", bufs=1))
    big = ctx.enter_context(tc.tile_pool(name="big", bufs=1))
    attn_sb = ctx.enter_context(tc.tile_pool(name="attn_sb", bufs=4))
    norm_sb = ctx.enter_context(tc.tile_pool(name="norm_sb", bufs=3))
    out_sb = ctx.enter_context(tc.tile_pool(name="out_sb", bufs=2))
    ps_st = ctx.enter_context(tc.tile_pool(name="ps_st", bufs=2, space="PSUM"))
    ps_av = ctx.enter_context(tc.tile_pool(name="ps_av", bufs=3, space="PSUM"))
    ps_bc = ctx.enter_context(tc.tile_pool(name="ps_bc", bufs=1, space="PSUM"))

    # ---- inputs -> SBUF (kT-projection inputs first so the PE starts early)
    wk_s = const.tile([128, DC, INNER], BF)
    wq_s = const.tile([128, DC, INNER], BF)
    wv_s = const.tile([128, DC, INNER], BF)
    wo_s = const.tile([128, EC, D_OUT], BF)
    bo_s = const.tile([1, D_OUT], F32)
    maskb_s = const.tile([128, KC], F32)
    xT_s = big.tile([128, DC, N], BF)
    xqT_s = big.tile([128, DC, NQ], BF)
    for d in range(DC):
        nc.sync.dma_start(wk_s[:, d, :], wkT[d])
        nc.sync.dma_start(xT_s[:, d, :], xT[d])
    for d in range(DC):
        nc.sync.dma_start(wq_s[:, d, :], wqT[d])
        nc.sync.dma_start(xqT_s[:, d, :], xqT[d])
        nc.sync.dma_start(wv_s[:, d, :], wvT[d])
    nc.sync.dma_start(maskb_s, maskb)
    for j in range(EC):
        nc.sync.dma_start(wo_s[:, j, :], woT[j])
    nc.sync.dma_start(bo_s, bo)

    ones_row = const.tile([1, 128], BF)
    nc.vector.memset(ones_row, 1.0)
    bo_bf = const.tile([1, D_OUT], BF)
    nc.vector.tensor_copy(bo_bf, bo_s)

    kT_s = big.tile([128, EC, N], BF)
    qT_s = big.tile([128, EC, NQ], BF)
    vp_s = big.tile([128, KC, H, DH + 1], BF)
    nc.vector.memset(vp_s[:, :, :, DH], 1.0)
    out_allT = big.tile([128, EC, NQ], BF)

    def proj_kT(j, b):
        ps = ps_av.tile([128, 512], F32, tag="av", name=f"ps_k{j}{b}")
        for d in range(DC):
            nc.tensor.matmul(
                ps, wk_s[:, d, j * 128:(j + 1) * 128],
                xT_s[:, d, b * 512:(b + 1) * 512],
                start=(d == 0), stop=(d == DC - 1))
        nc.vector.tensor_copy(kT_s[:, j, b * 512:(b + 1) * 512], ps)

    def proj_qT(j, b):
        ps = ps_av.tile([128, 512], F32, tag="av", name=f"ps_q{j}{b}")
        for d in range(DC):
            nc.tensor.matmul(
                ps, wq_s[:, d, j * 128:(j + 1) * 128],
                xqT_s[:, d, b * 512:(b + 1) * 512],
                start=(d == 0), stop=(d == DC - 1))
        nc.vector.tensor_copy(qT_s[:, j, b * 512:(b + 1) * 512], ps)

    def proj_v(c):
        ps = ps_av.tile([128, 512], F32, tag="av", name=f"ps_v{c}")
        for d in range(DC):
            nc.tensor.matmul(
                ps, xT_s[:, d, c * 128:(c + 1) * 128], wv_s[:, d, :],
                start=(d == 0), stop=(d == DC - 1))
        nc.vector.tensor_copy(
            vp_s[:, c, :, 0:DH],
            ps.rearrange("p (h e) -> p h e", h=H))

    # kT/qT for the first head pair, then v'; the rest interleaves below.
    for b in range(NKB):
        proj_kT(0, b)
    for b in range(NQB):
        proj_qT(0, b)

    fills = {p: [("k", p + 1, b) for b in range(NKB)]
                + [("q", p + 1, b) for b in range(NQB)]
             for p in range(EC - 1)}

    def emit_fill(p, it):
        lst = fills.get(p)
        if not lst:
            return
        # pair 0 also carries the v' projection in its first query block, so
        # its j=1 fills go in the second block only; other pairs spread their
        # 6 groups over all 32 chunk iterations.
        if p == 0:
            if it >= KC and it % 2 == 1 and lst:
                kind, j, b = lst.pop(0)
                (proj_kT if kind == "k" else proj_qT)(j, b)
        elif it % 5 == 4 and lst:
            kind, j, b = lst.pop(0)
            (proj_kT if kind == "k" else proj_qT)(j, b)

    for p in range(EC):
        hA, hB = 2 * p, 2 * p + 1
        for qb in range(NQB):
            oA = ps_av.tile([128, 512], F32, tag="av", name=f"oA{p}{qb}")
            oB = ps_av.tile([128, 512], F32, tag="av", name=f"oB{p}{qb}")
            for c in range(KC):
                if p == 0 and qb == 0:
                    proj_v(c)
                sT = ps_st.tile([128, 1024], F32, tag="st", name="sT")
                nc.tensor.matmul(
                    sT[:, 0:512],
                    kT_s[0:64, p, c * 128:(c + 1) * 128],
                    qT_s[0:64, p, qb * 512:(qb + 1) * 512],
                    start=True, stop=True)
                nc.tensor.matmul(
                    sT[:, 512:1024],
                    kT_s[64:128, p, c * 128:(c + 1) * 128],
                    qT_s[64:128, p, qb * 512:(qb + 1) * 512],
                    start=True, stop=True)
                at = attn_sb.tile([128, 1024], BF, tag="at", name="at")
                nc.scalar.activation(at, sT, EXP,
                                     bias=maskb_s[:, c:c + 1], scale=SCALE)
                nc.tensor.matmul(oA[0:DH + 1, :], vp_s[:, c, hA, :],
                                 at[:, 0:512],
                                 start=(c == 0), stop=(c == KC - 1))
                nc.tensor.matmul(oB[0:DH + 1, :], vp_s[:, c, hB, :],
                                 at[:, 512:1024],
                                 start=(c == 0), stop=(c == KC - 1))
                emit_fill(p, qb * KC + c)

            # normalize: out_allT[head rows] = out' * (1/denominator)
            recipA = norm_sb.tile([1, 512], F32, tag="recipA", name="recipA")
            recipB = norm_sb.tile([1, 512], F32, tag="recipB", name="recipB")
            nc.vector.reciprocal(recipA, oA[DH:DH + 1, :])
            nc.vector.reciprocal(recipB, oB[DH:DH + 1, :])
            recipAb = norm_sb.tile([1, 512], BF, tag="recipAb", name="recipAb")
            recipBb = norm_sb.tile([1, 512], BF, tag="recipBb", name="recipBb")
            nc.vector.tensor_copy(recipAb, recipA)
            nc.vector.tensor_copy(recipBb, recipB)
            bc = ps_bc.tile([128, 512], F32, tag="bc", name="bc")
            nc.tensor.matmul(bc[0:64, :], ones_row[0:1, 0:64], recipAb,
                             start=True, stop=True)
            nc.tensor.matmul(bc[64:128, :], ones_row[0:1, 0:64], recipBb,
                             start=True, stop=True)
            bc_s = norm_sb.tile([128, 512], F32, tag="bcs", name="bc_s")
            nc.vector.tensor_copy(bc_s, bc)
            nc.vector.tensor_mul(
                out_allT[0:64, p, qb * 512:(qb + 1) * 512],
                oA[0:DH, :], bc_s[0:64, :])
            nc.vector.tensor_mul(
                out_allT[64:128, p, qb * 512:(qb + 1) * 512],
                oB[0:DH, :], bc_s[64:128, :])

    # ---- output projection (+ bias via K=1 matmul)
    for t in range(NQ // 128):
        of = out_sb.tile([128, D_OUT], F32, tag="of", name="of")
        for f in range(D_OUT // 512):
            po = ps_av.tile([128, 512], F32, tag="av", name=f"po{t}{f}")
            nc.tensor.matmul(po, ones_row, bo_bf[0:1, f * 512:(f + 1) * 512],
                             start=True, stop=False)
            for j in range(EC):
                nc.tensor.matmul(
                    po, out_allT[:, j, t * 128:(t + 1) * 128],
                    wo_s[:, j, f * 512:(f + 1) * 512],
                    start=False, stop=(j == EC - 1))
            nc.vector.tensor_copy(of[:, f * 512:(f + 1) * 512], po)
        nc.sync.dma_start(out[t * 128:(t + 1) * 128, :], of)


def _build():
    nc = bacc.Bacc("TRN2", target_bir_lowering=False, debug=False,
                   num_devices=NCORES)
    aps = dict(
        xT=nc.dram_tensor("xT", [DC, 128, N], BF, kind="ExternalInput").ap(),
        xqT=nc.dram_tensor("xqT", [DC, 128, NQ], BF, kind="ExternalInput").ap(),
        wqT=nc.dram_tensor("wqT", [DC, 128, INNER], BF, kind="ExternalInput").ap(),
        wkT=nc.dram_tensor("wkT", [DC, 128, INNER], BF, kind="ExternalInput").ap(),
        wvT=nc.dram_tensor("wvT", [DC, 128, INNER], BF, kind="ExternalInput").ap(),
        woT=nc.dram_tensor("woT", [EC, 128, D_OUT], BF, kind="ExternalInput").ap(),
        bo=nc.dram_tensor("bo", [1, D_OUT], F32, kind="ExternalInput").ap(),
        maskb=nc.dram_tensor("maskb", [128, KC], F32, kind="ExternalInput").ap(),
        out=nc.dram_tensor("out", [NQ, D_OUT], F32, kind="ExternalOutput").ap(),
    )
    with tile.TileContext(nc) as tc:
        with ExitStack() as ctx:
            _emit(ctx, tc, **aps)
    nc.compile()
    return nc


_prog = None


def _get_prog():
    global _prog
    if _prog is None:
        _prog = _build()
    return _prog


def _make_in_maps(x, Wq, Wk, Wv, Wo, bo, mask):
    bf = ml_dtypes.bfloat16
    f32 = np.float32
    wqT = np.ascontiguousarray(Wq.T).astype(bf).reshape(DC, 128, INNER)
    wkT = np.ascontiguousarray(Wk.T).astype(bf).reshape(DC, 128, INNER)
    wvT = np.ascontiguousarray(Wv.T).astype(bf).reshape(DC, 128, INNER)
    woT = np.ascontiguousarray(Wo.T).astype(bf).reshape(EC, 128, D_OUT)
    bo2 = np.ascontiguousarray(bo).astype(f32).reshape(1, D_OUT)
    in_maps = []
    for c in range(NCORES):
        l, qh = c // 2, c % 2
        xTl = np.ascontiguousarray(x[l].T).astype(bf).reshape(DC, 128, N)
        xqT = np.ascontiguousarray(
            x[l, qh * NQ:(qh + 1) * NQ, :].T).astype(bf).reshape(DC, 128, NQ)
        mb = np.where(mask[l], 0.0, MASK_NEG).astype(f32)
        mb = np.ascontiguousarray(mb.reshape(KC, 128).T)
        in_maps.append(dict(xT=xTl, xqT=xqT, wqT=wqT, wkT=wkT, wvT=wvT,
                            woT=woT, bo=bo2, maskb=mb))
    return in_maps


def run(x, Wq, Wk, Wv, Wo, bo, mask, trace=False, tmpdir=None):
    nc = _get_prog()
    in_maps = _make_in_maps(x, Wq, Wk, Wv, Wo, bo, mask)
    res = run_bass_kernel_spmd(nc, in_maps, core_ids=list(range(NCORES)),
                               trace=trace, tmpdir=tmpdir)
    out = np.empty((L, N, D_OUT), np.float32)
    for c in range(NCORES):
        l, qh = c // 2, c % 2
        out[l, qh * NQ:(qh + 1) * NQ, :] = res.results[c]["out"]
    return out, res


def kernel(x, Wq, Wk, Wv, Wo, bo, mask):
    out, _ = run(np.asarray(x, np.float32), np.asarray(Wq, np.float32),
                 np.asarray(Wk, np.float32), np.asarray(Wv, np.float32),
                 np.asarray(Wo, np.float32), np.asarray(bo, np.float32),
                 np.asarray(mask))
    return out


# revision 14
# speedup vs baseline: 1.1640x; 1.0500x over previous
"""Cross-attention kernel for 8 Trainium2 NeuronCores (Bass/Tile).

Sharding: data-parallel over (L, query-half). Core c handles batch l = c//2
and queries [(c%2)*1024, (c%2+1)*1024) of that batch. K/V for the full 2048
keys of batch l are computed on both cores of the pair (duplicated work, no
cross-core communication).

Per-core dataflow (matmuls in bf16 with f32 PSUM accumulation):
  qT[e, nq], kT[e, nk]   transposed projections (feature dim on partitions)
  v'[nk, h, 65]          v projection + a ones column per head (softmax denom)
  sT[nk, nq] = kT.T @ qT per head  -- scores transposed, keys on partitions;
                                      head pairs run row-packed on the PE
  attnT = exp(SCALE*sT + maskbias[nk])  one ACT op fuses scale+mask+exp+cast
  out'[65, nq] = v'.T @ attnT      rows 0..63: head out^T, row 64: denominator
  normalize: DVE reciprocal of row 64, broadcast across partitions via a
             tiny ones-block matmul, DVE multiply
  out = out_allT.T @ WoT + bo      final projection, bias via a K=1 matmul
"""

import numpy as np
import ml_dtypes
from contextlib import ExitStack

import concourse.bass as bass
import concourse.tile as tile
from concourse import bacc, mybir
from concourse.bass_utils import run_bass_kernel_spmd

L, N, D_IN = 4, 2048, 1024
H, DH = 8, 64
INNER = H * DH          # 512
D_OUT = D_IN
SCALE = DH ** -0.5      # 0.125
NQ = N // 2             # 1024 queries per core
NCORES = 8
DC = D_IN // 128        # 8 contraction chunks for the projections
EC = INNER // 128       # 4 feature chunks (= head pairs)
KC = N // 128           # 16 key chunks
NKB = N // 512          # 4 key 512-blocks
NQB = NQ // 512         # 2 query 512-blocks
MASK_NEG = -50.0

BF = mybir.dt.bfloat16
F32 = mybir.dt.float32
EXP = mybir.ActivationFunctionType.Exp


def _emit(ctx, tc, xT, xqT, wqT, wkT, wvT, woT, bo, maskb, out):
    nc = tc.nc

    const = ctx.enter_context(tc.tile_pool(name="const", bufs=1))
    big = ctx.enter_context(tc.tile_pool(name="big", bufs=1))
    attn_sb = ctx.enter_context(tc.tile_pool(name="attn_sb", bufs=4))
    norm_sb = ctx.enter_context(tc.tile_pool(name="norm_sb", bufs=3))
    out_sb = ctx.enter_context(tc.tile_pool(name="out_sb", bufs=2))
    ps_st = ctx.enter_context(tc.tile_pool(name="ps_st", bufs=2, space="PSUM"))
    ps_av = ctx.enter_context(tc.tile_pool(name="ps_av", bufs=4, space="PSUM"))

    # ---- inputs -> SBUF (kT-projection inputs first so the PE starts early)
    wk_s = const.tile([128, DC, INNER], BF)
    wq_s = const.tile([128, DC, INNER], BF)
    wv_s = const.tile([128, DC, INNER], BF)
    wo_s = const.tile([128, EC, D_OUT], BF)
    bo_s = const.tile([1, D_OUT], F32)
    maskb_s = const.tile([128, KC], F32)
    xT_s = big.tile([128, DC, N], BF)
    xqT_s = big.tile([128, DC, NQ], BF)
    for d in range(DC):
        nc.sync.dma_start(wk_s[:, d, :], wkT[d])
        nc.sync.dma_start(xT_s[:, d, :], xT[d])
    for d in range(DC):
        nc.sync.dma_start(wq_s[:, d, :], wqT[d])
        nc.sync.dma_start(xqT_s[:, d, :], xqT[d])
        nc.sync.dma_start(wv_s[:, d, :], wvT[d])
    nc.sync.dma_start(maskb_s, maskb)
    for j in range(EC):
        nc.sync.dma_start(wo_s[:, j, :], woT[j])
    nc.sync.dma_start(bo_s, bo)

    ones_row = const.tile([1, 128], BF)
    nc.vector.memset(ones_row, 1.0)
    bo_bf = const.tile([1, D_OUT], BF)
    nc.vector.tensor_copy(bo_bf, bo_s)

    kT_s = big.tile([128, EC, N], BF)
    qT_s = big.tile([128, EC, NQ], BF)
    vp_s = big.tile([128, KC, H, DH + 1], BF)
    nc.vector.memset(vp_s[:, :, :, DH], 1.0)
    out_allT = big.tile([128, EC, NQ], BF)

    def proj_kT(j, b):
        ps = ps_av.tile([128, 512], F32, tag="av", name=f"ps_k{j}{b}")
        for d in range(DC):
            nc.tensor.matmul(
                ps, wk_s[:, d, j * 128:(j + 1) * 128],
                xT_s[:, d, b * 512:(b + 1) * 512],
                start=(d == 0), stop=(d == DC - 1))
        nc.vector.tensor_copy(kT_s[:, j, b * 512:(b + 1) * 512], ps)

    def proj_qT(j, b):
        ps = ps_av.tile([128, 512], F32, tag="av", name=f"ps_q{j}{b}")
        for d in range(DC):
            nc.tensor.matmul(
                ps, wq_s[:, d, j * 128:(j + 1) * 128],
                xqT_s[:, d, b * 512:(b + 1) * 512],
                start=(d == 0), stop=(d == DC - 1))
        nc.vector.tensor_copy(qT_s[:, j, b * 512:(b + 1) * 512], ps)

    def proj_v(c):
        ps = ps_av.tile([128, 512], F32, tag="av", name=f"ps_v{c}")
        for d in range(DC):
            nc.tensor.matmul(
                ps, xT_s[:, d, c * 128:(c + 1) * 128], wv_s[:, d, :],
                start=(d == 0), stop=(d == DC - 1))
        nc.vector.tensor_copy(
            vp_s[:, c, :, 0:DH],
            ps.rearrange("p (h e) -> p h e", h=H))

    # ---- warmup: junk matmuls to lift the PE HAM clock gate and a junk
    # exp to pull the ACT table load off the critical path, all during DMA.
    warm = const.tile([128, 512], BF)
    nc.vector.memset(warm, 1.0)
    wps = ps_av.tile([128, 512], F32, tag="av", name="wps")
    for i in range(18):
        nc.tensor.matmul(wps, warm[:, 0:128], warm, start=(i == 0),
                         stop=(i == 17))
    warm_out = const.tile([1, 32], BF)
    nc.scalar.activation(warm_out, wps[0:1, 0:32], EXP, bias=0.0, scale=0.0)

    # kT/qT for the first two head pairs up front; later pairs fill PE gaps
    # in the ACT-bound attention stretches.
    for j in range(2):
        for b in range(NKB):
            proj_kT(j, b)
        for b in range(NQB):
            proj_qT(j, b)

    def normalize(p, qb, oA, oB):
        # out_allT[head rows] = out' * (1/denominator)
        recipA = norm_sb.tile([1, 512], F32, tag="recipA", name="recipA")
        recipB = norm_sb.tile([1, 512], F32, tag="recipB", name="recipB")
        nc.vector.reciprocal(recipA, oA[DH:DH + 1, :])
        nc.vector.reciprocal(recipB, oB[DH:DH + 1, :])
        recipAb = norm_sb.tile([1, 512], BF, tag="recipAb", name="recipAb")
        recipBb = norm_sb.tile([1, 512], BF, tag="recipBb", name="recipBb")
        nc.vector.tensor_copy(recipAb, recipA)
        nc.vector.tensor_copy(recipBb, recipB)
        # broadcast tiles borrow an st-pool slot (freed at exp pace, and the
        # exp chain never depends on normalize, so no slot deadlock)
        bc = ps_st.tile([128, 512], F32, tag="st", name="bc")
        nc.tensor.matmul(bc[0:64, :], ones_row[0:1, 0:64], recipAb,
                         start=True, stop=True)
        nc.tensor.matmul(bc[64:128, :], ones_row[0:1, 0:64], recipBb,
                         start=True, stop=True)
        bc_s = norm_sb.tile([128, 512], F32, tag="bcs", name="bc_s")
        nc.vector.tensor_copy(bc_s, bc)
        nc.vector.tensor_mul(
            out_allT[0:64, p, qb * 512:(qb + 1) * 512],
            oA[0:DH, :], bc_s[0:64, :])
        nc.vector.tensor_mul(
            out_allT[64:128, p, qb * 512:(qb + 1) * 512],
            oB[0:DH, :], bc_s[64:128, :])

    def outproj_t(t):
        of = out_sb.tile([128, D_OUT], F32, tag="of", name="of")
        for f in range(D_OUT // 512):
            po = ps_av.tile([128, 512], F32, tag="av", name=f"po{t}{f}")
            nc.tensor.matmul(po, ones_row, bo_bf[0:1, f * 512:(f + 1) * 512],
                             start=True, stop=False)
            for j in range(EC):
                nc.tensor.matmul(
                    po, out_allT[:, j, t * 128:(t + 1) * 128],
                    wo_s[:, j, f * 512:(f + 1) * 512],
                    start=False, stop=(j == EC - 1))
            nc.vector.tensor_copy(of[:, f * 512:(f + 1) * 512], po)
        nc.sync.dma_start(out[t * 128:(t + 1) * 128, :], of)

    # work queues drained inside the attention chunk loops
    pending_norm = []   # deferred normalize closures (emit a few chunks late)
    fill_q = []         # projection / outproj groups to slot into PE gaps

    # attention sweep: query-block outer, head-pair inner. The first sweep
    # (qb=0) interleaves the v' projection (pair 0) and later pairs' kT/qT
    # projections; the second sweep interleaves the first half of the output
    # projection, whose nq tiles only need qb=0 columns of out_allT.
    for qb in range(NQB):
        for p in range(EC):
            hA, hB = 2 * p, 2 * p + 1
            if qb == 0 and 1 <= p < EC - 1:
                # pair p+2's projections drain during pair p+1's chunks (the
                # queue is popped with one block of lag), comfortably before
                # pair p+2's first score matmul reads them
                fill_q.extend([("k", p + 1, b) for b in range(NKB)]
                              + [("q", p + 1, b) for b in range(NQB)])
            if qb == 1 and p < 2:
                fill_q.extend([("o", 2 * p + i, None) for i in range(2)])
            oA = ps_av.tile([128, 512], F32, tag="av", name=f"oA{p}{qb}")
            oB = ps_av.tile([128, 512], F32, tag="av", name=f"oB{p}{qb}")
            for c in range(KC):
                sT = ps_st.tile([128, 1024], F32, tag="st", name="sT")
                nc.tensor.matmul(
                    sT[:, 0:512],
                    kT_s[0:64, p, c * 128:(c + 1) * 128],
                    qT_s[0:64, p, qb * 512:(qb + 1) * 512],
                    start=True, stop=True)
                nc.tensor.matmul(
                    sT[:, 512:1024],
                    kT_s[64:128, p, c * 128:(c + 1) * 128],
                    qT_s[64:128, p, qb * 512:(qb + 1) * 512],
                    start=True, stop=True)
                at = attn_sb.tile([128, 1024], BF, tag="at", name="at")
                nc.scalar.activation(at, sT, EXP,
                                     bias=maskb_s[:, c:c + 1], scale=SCALE)
                if qb == 0 and p == 0:
                    proj_v(c)
                if c == 2 and pending_norm:
                    pending_norm.pop(0)()
                nc.tensor.matmul(oA[0:DH + 1, :], vp_s[:, c, hA, :],
                                 at[:, 0:512],
                                 start=(c == 0), stop=(c == KC - 1))
                nc.tensor.matmul(oB[0:DH + 1, :], vp_s[:, c, hB, :],
                                 at[:, 512:1024],
                                 start=(c == 0), stop=(c == KC - 1))
                if fill_q and not (qb == 0 and p == 0) and c >= 3 and c % 2 == 1:
                    kind, j, b = fill_q.pop(0)
                    if kind == "k":
                        proj_kT(j, b)
                    elif kind == "q":
                        proj_qT(j, b)
                    else:
                        outproj_t(j)
            pending_norm.append(
                lambda p=p, qb=qb, oA=oA, oB=oB: normalize(p, qb, oA, oB))

    while pending_norm:
        pending_norm.pop(0)()
    while fill_q:
        kind, j, b = fill_q.pop(0)
        if kind == "k":
            proj_kT(j, b)
        elif kind == "q":
            proj_qT(j, b)
        else:
            outproj_t(j)
    # ---- remaining output projection (nq tiles needing qb=1 columns)
    for t in range(NQ // 256, NQ // 128):
        outproj_t(t)


def _build():
    nc = bacc.Bacc("TRN2", target_bir_lowering=False, debug=False,
                   num_devices=NCORES)
    aps = dict(
        xT=nc.dram_tensor("xT", [DC, 128, N], BF, kind="ExternalInput").ap(),
        xqT=nc.dram_tensor("xqT", [DC, 128, NQ], BF, kind="ExternalInput").ap(),
        wqT=nc.dram_tensor("wqT", [DC, 128, INNER], BF, kind="ExternalInput").ap(),
        wkT=nc.dram_tensor("wkT", [DC, 128, INNER], BF, kind="ExternalInput").ap(),
        wvT=nc.dram_tensor("wvT", [DC, 128, INNER], BF, kind="ExternalInput").ap(),
        woT=nc.dram_tensor("woT", [EC, 128, D_OUT], BF, kind="ExternalInput").ap(),
        bo=nc.dram_tensor("bo", [1, D_OUT], F32, kind="ExternalInput").ap(),
        maskb=nc.dram_tensor("maskb", [128, KC], F32, kind="ExternalInput").ap(),
        out=nc.dram_tensor("out", [NQ, D_OUT], F32, kind="ExternalOutput").ap(),
    )
    with tile.TileContext(nc) as tc:
        with ExitStack() as ctx:
            _emit(ctx, tc, **aps)
    nc.compile()
    return nc


_prog = None


def _get_prog():
    global _prog
    if _prog is None:
        _prog = _build()
    return _prog


def _make_in_maps(x, Wq, Wk, Wv, Wo, bo, mask):
    bf = ml_dtypes.bfloat16
    f32 = np.float32
    wqT = np.ascontiguousarray(Wq.T).astype(bf).reshape(DC, 128, INNER)
    wkT = np.ascontiguousarray(Wk.T).astype(bf).reshape(DC, 128, INNER)
    wvT = np.ascontiguousarray(Wv.T).astype(bf).reshape(DC, 128, INNER)
    woT = np.ascontiguousarray(Wo.T).astype(bf).reshape(EC, 128, D_OUT)
    bo2 = np.ascontiguousarray(bo).astype(f32).reshape(1, D_OUT)
    in_maps = []
    for c in range(NCORES):
        l, qh = c // 2, c % 2
        xTl = np.ascontiguousarray(x[l].T).astype(bf).reshape(DC, 128, N)
        xqT = np.ascontiguousarray(
            x[l, qh * NQ:(qh + 1) * NQ, :].T).astype(bf).reshape(DC, 128, NQ)
        mb = np.where(mask[l], 0.0, MASK_NEG).astype(f32)
        mb = np.ascontiguousarray(mb.reshape(KC, 128).T)
        in_maps.append(dict(xT=xTl, xqT=xqT, wqT=wqT, wkT=wkT, wvT=wvT,
                            woT=woT, bo=bo2, maskb=mb))
    return in_maps


def run(x, Wq, Wk, Wv, Wo, bo, mask, trace=False, tmpdir=None):
    nc = _get_prog()
    in_maps = _make_in_maps(x, Wq, Wk, Wv, Wo, bo, mask)
    res = run_bass_kernel_spmd(nc, in_maps, core_ids=list(range(NCORES)),
                               trace=trace, tmpdir=tmpdir)
    out = np.empty((L, N, D_OUT), np.float32)
    for c in range(NCORES):
        l, qh = c // 2, c % 2
        out[l, qh * NQ:(qh + 1) * NQ, :] = res.results[c]["out"]
    return out, res


def kernel(x, Wq, Wk, Wv, Wo, bo, mask):
    out, _ = run(np.asarray(x, np.float32), np.asarray(Wq, np.float32),
                 np.asarray(Wk, np.float32), np.asarray(Wv, np.float32),
                 np.asarray(Wo, np.float32), np.asarray(bo, np.float32),
                 np.asarray(mask))
    return out
